# revision 37
# baseline (speedup 1.0000x reference)
"""Bass/Tile TRN2 kernel for nn_Attention (Bahdanau-style attention scores).

Computation (per batch b):
    energy[s, h] = tanh( (enc[b] @ We)[s, h] + (hidden[b] @ Wh)[h] + bias[h] )
    scores[s]    = sum_h energy[s, h] * v[h]
    out[b]       = softmax(scores)

Sharding: data-parallel over batch B=32 across 8 cores (4 batches/core);
weights replicated.

v5 design (head/tail restructure around the bf16 GEMM roofline; fp8
DoubleRow measures the same 222ns/instr as bf16 so it cannot win):
  - DMA: all head granules use >=4KB per-partition lines (per-engine DMA
    throughput is packet-bound: 2KB lines halve bandwidth).  Order is
    critical-first: tab (3KB), enc b0 st0-1, We j-quarters, enc b0 st2-3,
    st4-7, enc b1-3.  The first two groups accumulate j-quarter by
    j-quarter so the GEMM starts once ~770KB have landed.
  - hb (h_proj+bias) and v are NOT shipped as 128-row broadcast tables
    (768KB); instead a 3KB 5-partition tab is broadcast on-chip with five
    [5,128]x[5,512] PE matmuls (which double as HAM warm-up) + DVE
    copies.
  - PE warm-up on the identity tile (f32 matmul = 4cyc/row) keeps the
    HAM clock at 2.4GHz from the first real matmul.
  - Tail: per-batch softmax normalization overlapped under the next
    batch's GEMM; for the last batch the hb-add is folded into the GEMM
    as a 9th matmul (stationary = row0-ones, moving = broadcast hb), the
    tanh reads PSUM directly, the tiny norm matmuls run in f32r (single
    pass), and dummy matmuls keep the PE p-state high through the chain.
"""

import ml_dtypes
import numpy as np

import concourse.bass as bass
import concourse.tile as tile
from concourse import bacc, mybir
from concourse import bass_utils
from concourse.masks import make_identity

F32 = mybir.dt.float32
F32R = mybir.dt.float32r
BF16 = mybir.dt.bfloat16
AFT = mybir.ActivationFunctionType
ALU = mybir.AluOpType

N_CORES = 8
B = 32
B_LOC = B // N_CORES  # 4
S = 1024
H = 512
E2 = 2 * H  # 1024
P = 128
N_ET = E2 // P   # 8 e-tiles (contraction)
N_ST = S // P    # 8 s-tiles per batch
TABW = 1664      # 512 (hb/v) + 5*128 (bcast tables) + 64 (sel) + pad
N_WARM1 = 6      # full-K warm-ups: ramp the HAM clock before the bcasts
N_WARM2 = 2      # warm-ups bridging bcast -> first GEMM matmul
N_DUM = 10       # tail dummies keeping the PE p-state high


def build():
    nc = bacc.Bacc("TRN2", target_bir_lowering=False, debug=False)
    # host layout: enc[b, ep, st, j, sp] = encT[b, j*128+ep, st*128+sp]
    enc_d = nc.dram_tensor(
        "enc", [B_LOC, P, N_ST, N_ET, P], BF16, kind="ExternalInput"
    ).ap()
    # host layout: We[ep, j, h] = We[j*128+ep, h]
    We_d = nc.dram_tensor("We", [P, N_ET, H], BF16, kind="ExternalInput").ap()
    # tab[0:4, 0:512]=hb rows, tab[4, 0:512]=v, tab[p, 512+128t+m]=(p==t),
    # tab[b, 1152:1216]=f32 selection row (bitcast) for recip broadcast
    tab_d = nc.dram_tensor("tab", [5, TABW], BF16, kind="ExternalInput").ap()
    out_d = nc.dram_tensor("out", [B_LOC, N_ST, P], F32, kind="ExternalOutput").ap()

    with tile.TileContext(nc) as tc:
        with (
            tc.tile_pool(name="consts", bufs=1) as consts,
            tc.tile_pool(name="t1p", bufs=3) as t1p,
            tc.tile_pool(name="enp", bufs=3) as enp,
            tc.tile_pool(name="zps", bufs=6, space="PSUM") as zps,
            tc.tile_pool(name="eps", bufs=1, space="PSUM") as eps,
            tc.tile_pool(name="softp", bufs=1, space="PSUM") as softp,
        ):
            # ---- warm tile (Vector memset, ready ~6.5us) ----
            wm = consts.tile([P, H], BF16, name="warm")
            nc.vector.memset(wm[:], 0.0)
            # identity (GpSimd): used for tail transposes + dummies
            ident = consts.tile([P, P], F32)
            make_identity(nc, ident[:])

            # ---- HAM warm-up round 1 ----
            zw = zps.tile([P, H], F32, tag="z", name="zwarm")
            for i in range(N_WARM1):
                nc.tensor.matmul(
                    zw[:], wm[:, :P], wm[:],
                    start=(i == 0), stop=(i == N_WARM1 - 1),
                )

            # ---- DMA stream (sync HWDGE ring, need-order, FIFO-biased).
            # Few, single DMAs win: each descriptor-gen occupies the Sync
            # sequencer ~0.65us, so extra streams delay the enc batch DMAs
            # and starve the mid-GEMM (measured). We j-quarters let the
            # first groups start accumulating as soon as ~770KB landed. ----
            tab_sb = consts.tile([5, TABW], BF16, name="tab")
            nc.sync.dma_start(tab_sb[:], tab_d)
            e0_t01 = consts.tile([P, 2, N_ET, P], BF16, name="enc0_st01")
            nc.sync.dma_start(e0_t01[:], enc_d[0, :, 0:2])
            We_q = []
            for q in range(4):
                t = consts.tile([P, 2, H], BF16, name=f"We_q{q}")
                nc.sync.dma_start(t[:], We_d[:, 2 * q:2 * q + 2, :])
                We_q.append(t)
            e0_t23 = consts.tile([P, 2, N_ET, P], BF16, name="enc0_st23")
            nc.sync.dma_start(e0_t23[:], enc_d[0, :, 2:4])
            e0_t45 = consts.tile([P, 2, N_ET, P], BF16, name="enc0_st45")
            nc.sync.dma_start(e0_t45[:], enc_d[0, :, 4:6])
            e0_t67 = consts.tile([P, 2, N_ET, P], BF16, name="enc0_st67")
            nc.sync.dma_start(e0_t67[:], enc_d[0, :, 6:8])
            enc_b = {}
            for b in range(1, B_LOC):
                t = consts.tile([P, N_ST, N_ET, P], BF16, name=f"enc{b}")
                nc.sync.dma_start(t[:], enc_d[b])
                enc_b[b] = t

            def enc_ap(b, st):
                if b == 0:
                    return (e0_t01, e0_t23, e0_t45, e0_t67)[st // 2][:, st % 2]
                return enc_b[b][:, st]

            # ---- on-chip broadcast of hb rows + v (also warms the PE) ----
            sm_sb = consts.tile([P, B_LOC + 1, H], BF16, name="sm_sb")
            bc_z = []
            for i in range(B_LOC + 1):
                zb = zps.tile([P, H], F32, tag="z", name=f"bc{i}")
                nc.tensor.matmul(
                    zb[:],
                    tab_sb[0:5, 512 + P * i:512 + P * (i + 1)],
                    tab_sb[0:5, 0:H],
                    start=True, stop=True,
                )
                bc_z.append(zb)
            for i in (0, B_LOC, 1, 2, 3):  # hb0 and v first (needed soonest)
                nc.vector.tensor_copy(sm_sb[:, i, :], bc_z[i][:])

            # ---- HAM warm-up round 2 (bridges bcast -> first real mm) ----
            if N_WARM2:
                zw2 = zps.tile([P, H], F32, tag="z", name="zwarm2")
                for i in range(N_WARM2):
                    nc.tensor.matmul(
                        zw2[:], wm[:, :P], wm[:],
                        start=(i == 0), stop=(i == N_WARM2 - 1),
                    )

            # ---- small consts / working tiles (softmax chain in bf16:
            # single-pass tiny matmuls instead of fp32 double passes) ----
            ones_col = consts.tile([P, 1], BF16)
            nc.vector.memset(ones_col[:], 1.0)
            rowsum = consts.tile([P, B_LOC], BF16, name="rowsum")
            nc.vector.memset(rowsum[:], 1.0)  # finite values in unused rows
            r0ones = consts.tile([P, P], BF16, name="r0ones")
            nc.vector.memset(r0ones[:], 0.0)
            nc.vector.memset(r0ones[0:1, :], 1.0)
            scores_all = consts.tile([P, B_LOC * N_ST], F32, name="scores")
            exp_all = consts.tile([P, B_LOC * N_ST], BF16, name="exp")
            scrap = consts.tile([P, H], BF16, name="stt_scrap")
            rec4_sb = consts.tile([B_LOC, 1], BF16, name="rec4")
            sel_ap = tab_sb[0:B_LOC, 1152:1152 + B_LOC * N_ST]

            def group_epilogue(b, st, z):
                col = b * N_ST + st
                if not (b == B_LOC - 1 and st == N_ST - 1):
                    t1 = t1p.tile([P, H], BF16, tag="t1")
                    nc.vector.tensor_tensor(t1[:], z[:], sm_sb[:, b, :], ALU.add)
                    en = enp.tile([P, H], BF16, tag="en")
                    nc.scalar.activation(en[:], t1[:], AFT.Tanh)
                    # fused v-dot: scrap = en * v, accum = sum over h
                    nc.vector.scalar_tensor_tensor(
                        scrap[:], en[:], 1.0, sm_sb[:, B_LOC, :],
                        op0=ALU.mult, op1=ALU.mult,
                        accum_out=scores_all[:, col:col + 1],
                    )
                else:
                    # final group: hb already folded into z by the 9th
                    # matmul; tanh reads PSUM; h-halves pipeline DVE/ScalarE
                    HH = H // 2
                    pparts = []
                    for h0 in (0, HH):
                        en = enp.tile([P, HH], BF16, tag="enh")
                        nc.scalar.activation(en[:], z[:, h0:h0 + HH], AFT.Tanh)
                        pacc = enp.tile([P, 1], F32, tag=f"pac{h0}")
                        nc.vector.scalar_tensor_tensor(
                            scrap[:, :HH], en[:], 1.0,
                            sm_sb[:, B_LOC, h0:h0 + HH],
                            op0=ALU.mult, op1=ALU.mult, accum_out=pacc[:],
                        )
                        pparts.append(pacc)
                    nc.vector.tensor_tensor(
                        scores_all[:, col:col + 1],
                        pparts[0][:], pparts[1][:], ALU.add,
                    )

            def batch_exp(b):
                # ScalarE accumulates fp32 internally; only the write-out is
                # bf16 (keeps the per-batch total matmul single-pass)
                with nc.allow_low_precision(reason="bf16 rowsum write"):
                    nc.scalar.activation(
                        exp_all[:, b * N_ST:(b + 1) * N_ST],
                        scores_all[:, b * N_ST:(b + 1) * N_ST],
                        AFT.Exp, accum_out=rowsum[:, b:b + 1],
                    )

            expT_of = {}

            ident_bf = consts.tile([P, P], BF16, name="ident_bf")
            nc.gpsimd.tensor_copy(ident_bf[:], ident[:])

            def norm_a(b):
                # transpose exp block + per-batch total (PE, tiny)
                r0 = b * N_ST
                expT = eps.tile([N_ST, P], BF16, tag="expT", name=f"expT{b}")
                expT_of[b] = expT
                with nc.allow_low_precision(reason="bf16 softmax norm chain"):
                    nc.tensor.transpose(expT[:], exp_all[:, r0:r0 + N_ST],
                                        ident_bf[:])
                    nrm = softp.tile([N_ST, 2], F32, tag="nrm", name=f"nrm{b}")
                    nc.tensor.matmul(
                        nrm[0:B_LOC, 0:1], rowsum[:], ones_col[:],
                        start=True, stop=True,
                    )
                    nc.vector.reciprocal(rec4_sb[:], nrm[0:B_LOC, 0:1])
                return nrm

            def norm_b(b, nrm):
                # broadcast recip to the batch's 8 rows, scale, DMA out
                r0 = b * N_ST
                nc.tensor.matmul(
                    nrm[:, 1:2], sel_ap[:, r0:r0 + N_ST], rec4_sb[:],
                    start=True, stop=True,
                )
                rrep8 = t1p.tile([N_ST, 1], F32, tag="rrep8")
                nc.vector.tensor_copy(rrep8[:], nrm[:, 1:2])
                pt = t1p.tile([N_ST, P], F32, tag="pt")
                nc.vector.tensor_scalar_mul(pt[:], expT_of[b][:], rrep8[:])
                nc.sync.dma_start(out_d[b], pt[:])

            # ---- main loop: j-quarter split for (0,st0)/(0,st1) ----
            z_of = {}
            nrm_of = {}
            for g in range(2):
                z_of[g] = zps.tile([P, H], F32, tag="z", name=f"zhead{g}")
            for jq in range(4):
                for g in range(2):
                    for j in (2 * jq, 2 * jq + 1):
                        nc.tensor.matmul(
                            z_of[g][:], enc_ap(0, g)[:, j, :],
                            We_q[jq][:, j % 2, :],
                            start=(j == 0), stop=(j == N_ET - 1),
                        )
            for g in range(2):
                group_epilogue(0, g, z_of[g])

            for b in range(B_LOC):
                for st in range(N_ST):
                    if b == 0 and st < 2:
                        continue
                    last = (b == B_LOC - 1 and st == N_ST - 1)
                    z = zps.tile([P, H], F32, tag="z")
                    for j in range(N_ET):
                        nc.tensor.matmul(
                            z[:], enc_ap(b, st)[:, j, :],
                            We_q[j // 2][:, j % 2, :],
                            start=(j == 0), stop=(j == N_ET - 1 and not last),
                        )
                    if last:
                        # fold the hb add into the GEMM: += row0ones.T @ hb3
                        nc.tensor.matmul(
                            z[:], r0ones[:], sm_sb[:, B_LOC - 1, :],
                            start=False, stop=True,
                        )
                    group_epilogue(b, st, z)
                    # overlapped per-batch normalization of batch b-1
                    if b > 0 and st == 1:
                        nrm_of[b - 1] = norm_a(b - 1)
                    if b > 0 and st == 3:
                        norm_b(b - 1, nrm_of[b - 1])
                batch_exp(b)

            # tail: dummies keep the PE p-state high through the b3 chain
            zd = zps.tile([P, H], F32, tag="z", name="zdum")
            for i in range(N_DUM):
                nc.tensor.matmul(
                    zd[:, :P], ident[:], ident[:],
                    start=(i == 0), stop=(i == N_DUM - 1),
                )
            nrm3 = norm_a(B_LOC - 1)
            norm_b(B_LOC - 1, nrm3)

    nc.compile()
    return nc


_NC_CACHE = None


def _get_nc():
    global _NC_CACHE
    if _NC_CACHE is None:
        _NC_CACHE = build()
    return _NC_CACHE


def prep_in_maps(inputs):
    hidden = np.ascontiguousarray(np.asarray(inputs["hidden"], dtype=np.float32))
    enc = np.asarray(inputs["encoder_outputs"], dtype=np.float32)
    W = np.ascontiguousarray(np.asarray(inputs["W"], dtype=np.float32))
    b = np.ascontiguousarray(np.asarray(inputs["b"], dtype=np.float32))
    v = np.ascontiguousarray(np.asarray(inputs["v"], dtype=np.float32))

    bf16 = ml_dtypes.bfloat16
    # We[ep, j, h] layout
    We_bf = np.ascontiguousarray(
        W[H:].astype(bf16).reshape(N_ET, P, H).transpose(1, 0, 2)
    )
    # hb[b, h] = hidden @ Wh + bias  (tiny: 0.4% of total flops)
    hb = (hidden @ W[:H] + b).astype(bf16)  # [B, H]
    v_bf = v.astype(bf16)

    # enc[b, s, e] -> X[b, ep, st, j, sp] = encT layout
    enc_bf = enc.astype(bf16)  # [B, S, E2]
    X = np.ascontiguousarray(
        enc_bf.reshape(B, N_ST, P, N_ET, P).transpose(0, 4, 1, 3, 2)
    )  # [B, P, N_ST, N_ET, P]

    in_maps = []
    for c in range(N_CORES):
        lo, hi = c * B_LOC, (c + 1) * B_LOC
        tab = np.zeros((5, TABW), dtype=bf16)
        tab[:B_LOC, 0:H] = hb[lo:hi]
        tab[B_LOC, 0:H] = v_bf
        for t in range(B_LOC + 1):
            tab[t, H + P * t:H + P * (t + 1)] = 1.0
        for bb in range(B_LOC):
            tab[bb, 1152 + bb * N_ST:1152 + (bb + 1) * N_ST] = 1.0
        in_maps.append(
            {
                "enc": X[lo:hi],
                "We": We_bf,
                "tab": np.ascontiguousarray(tab),
            }
        )
    return in_maps


def run(inputs, trace=False, trace_kwargs=None):
    in_maps = prep_in_maps(inputs)
    nc = _get_nc()
    res = bass_utils.run_bass_kernel_spmd(
        nc,
        in_maps,
        core_ids=list(range(N_CORES)),
        trace=trace,
        **(trace_kwargs or {}),
    )
    full = np.concatenate(
        [res.results[c]["out"].reshape(B_LOC, S) for c in range(N_CORES)], axis=0
    )
    return full, res


def kernel(**inputs) -> np.ndarray:
    full, _ = run(inputs, trace=False)
    return full
